# revision 36
# baseline (speedup 1.0000x reference)
"""Trainium2 Bass kernel for nn_CausalAttention_56873956934253.

Causal attention, B=8, S=1024 (32x32), C=512, 8 heads, D=64, with
weight-normalized QKV projections (PyTorch weight_norm dim=0 style).

Sharding: pure data parallelism over batch - core b handles batch b.
Weights replicated. No collectives.

Per-core dataflow (all shapes hardcoded):
  WT prep:  pure PE transposes (fp32r is_transpose, 1.5 cyc/row) of the three
            natural [C, KC] weights into [KC%128, 3, C] layout.  The
            weight-norm scale g/||v|| is NOT applied to the weights:
            - Q, K: scale+bias folded into the PSUM->SBUF eviction of the
              projections (ACT Identity with per-partition scale/bias APs),
              output directly in bf16.
            - V: scale folded into the final normalization multiply
              (scalar_tensor_tensor per-partition scalar), bias pre-divided
              by the scale and added to raw V via a K=1 ones matmul.
  norms:    fused square+reduce (tensor_tensor_reduce) + exact sqrt/recip
            with one Newton refinement.
  QT/KT:    [C, S] layout bf16 (PE fp32r matmuls over 3 K-chunks).
  Yv:       [S, C] layout bf16, heads strided by 65 with a ones column
            (AV matmul then also produces softmax denominators).
  Scores:   bf16, per head pair on disjoint 64-row PE groups, emitted as
            adjacent pairs with 1-step software pipelining against the
            AV matmuls.  exp(s/8) fused into ACT eviction; strictly-upper
            diag blocks masked multiplicatively post-exp (DVE for half 0,
            GPSIMD for half 1).
  AV:       bf16, V stationary, accumulating po[65, q]; row 64 = softmax
            denominators.  Denominator row -> guard-max to SBUF -> K=1
            broadcast matmul into partitions 64:128 of the SAME po bank
            (no extra PSUM) -> reciprocal_approx_fast -> fused
            (po * v_scale) * recip normalize -> DMA out per (head, j).
  DMA:      inputs spread over 5 queues (sync/scalar/tensor/vector/gpsimd).
  Output [C, S] per core -> [8, 512, 32, 32].
"""

import numpy as np
from contextlib import ExitStack

import concourse.bacc as bacc
import concourse.bass as bass
import concourse.tile as tile
import concourse.mybir as mybir
from concourse.bass_utils import run_bass_kernel_spmd
from concourse.masks import make_identity, make_upper_triangular

P = 128
S = 1024
C = 512
KC = 384
NH = 8
D = 64
NB = 8  # batch == cores

F32 = mybir.dt.float32
F32R = mybir.dt.float32r
BF16 = mybir.dt.bfloat16

I16 = mybir.dt.int16
# fast-exp constants: exp(s/8) ~= bf16_frombits(round(s*A + B))
EXP_A = 0.125 * 1.4426950408889634 * 128.0
EXP_B = 16256.0 - 5.5

AF = mybir.ActivationFunctionType
ALU = mybir.AluOpType
AX = mybir.AxisListType


def _r(ap):
    return ap.bitcast(F32R)


def build_nc():
    nc = bacc.Bacc("TRN2", target_bir_lowering=False, debug=False)

    xq_d = nc.dram_tensor("xq", [KC, S], F32R, kind="ExternalInput")
    xk_d = nc.dram_tensor("xk", [KC, S], F32R, kind="ExternalInput")
    wq_d = nc.dram_tensor("wq", [C, KC], F32R, kind="ExternalInput")
    wk_d = nc.dram_tensor("wk", [C, KC], F32R, kind="ExternalInput")
    wv_d = nc.dram_tensor("wv", [C, KC], F32R, kind="ExternalInput")
    gq_d = nc.dram_tensor("gq", [C], F32, kind="ExternalInput")
    gk_d = nc.dram_tensor("gk", [C], F32, kind="ExternalInput")
    gv_d = nc.dram_tensor("gv", [C], F32, kind="ExternalInput")
    bq_d = nc.dram_tensor("bq", [C], F32, kind="ExternalInput")
    bk_d = nc.dram_tensor("bk", [C], F32, kind="ExternalInput")
    bv_d = nc.dram_tensor("bv", [C], F32, kind="ExternalInput")
    out_d = nc.dram_tensor("out", [C, S], F32, kind="ExternalOutput")

    with tile.TileContext(nc) as tc:
        with ExitStack() as ctx:
            _body(ctx, tc, xq_d, xk_d,
                  (wq_d, gq_d, bq_d), (wk_d, gk_d, bk_d), (wv_d, gv_d, bv_d),
                  out_d)
    nc.compile()
    return nc


def _body(ctx, tc, xq_d, xk_d, wq3, wk3, wv3, out_d):
    nc = tc.nc

    singles = ctx.enter_context(tc.tile_pool(name="singles", bufs=1))
    tmp = ctx.enter_context(tc.tile_pool(name="tmp", bufs=2))
    ps_s = ctx.enter_context(tc.tile_pool(name="ps_s", bufs=3, space="PSUM"))
    ps_x = ctx.enter_context(tc.tile_pool(name="ps_x", bufs=2, space="PSUM"))
    ps_o = ctx.enter_context(tc.tile_pool(name="ps_o", bufs=3, space="PSUM"))
    es_pool = ctx.enter_context(tc.tile_pool(name="es", bufs=28))
    out_pool = ctx.enter_context(tc.tile_pool(name="outp", bufs=4))
    small = ctx.enter_context(tc.tile_pool(name="small", bufs=8))

    wq_d, gq_d, bq_d = wq3
    wk_d, gk_d, bk_d = wk3
    wv_d, gv_d, bv_d = wv3

    # identity first: it is produced on GPSIMD and the first PE transpose
    # paces on the GPSIMD instruction counter, so it must precede the
    # (slow) DMA-issue instructions on that queue.
    ident = singles.tile([P, P], F32, tag="ident")
    make_identity(nc, ident)
    identr = singles.tile([P, P], F32R, tag="identr")
    nc.vector.tensor_copy(identr, ident)

    # ---------------- input DMAs, spread across the 5 engine queues
    w_nat_q = singles.tile([P, 4, KC], F32R, tag="wnq")
    w_nat_k = singles.tile([P, 4, KC], F32R, tag="wnk")
    w_nat_v = singles.tile([P, 4, KC], F32R, tag="wnv")
    xq_s = singles.tile([P, 3, S], F32R, tag="xq_s")
    xk_s = singles.tile([P, 3, S], F32R, tag="xk_s")
    # per-channel vectors in [128, 4] column layout (c = g*128 + p)
    gq_col = singles.tile([P, 4], F32, tag="gq")
    bq_col = singles.tile([P, 4], F32, tag="bq")
    gk_col = singles.tile([P, 4], F32, tag="gk")
    bk_col = singles.tile([P, 4], F32, tag="bk")
    gv_col = singles.tile([P, 4], F32, tag="gv")
    bv_col = singles.tile([P, 4], F32, tag="bv")

    # tiny gathers first (~1us each while HBM is still quiet), then the big
    # streams; wq gets the full early bandwidth on the sync queue.
    for g in range(4):
        nc.sync.dma_start(out=w_nat_q[:, g, :], in_=wq_d.ap()[g * P:(g + 1) * P, :])
    for g in range(4):
        nc.scalar.dma_start(out=w_nat_k[:, g, :], in_=wk_d.ap()[g * P:(g + 1) * P, :])
    for g in range(4):
        nc.gpsimd.dma_start(out=w_nat_v[:, g, :], in_=wv_d.ap()[g * P:(g + 1) * P, :])
    nc.scalar.dma_start(out=gq_col, in_=gq_d.ap().rearrange("(g p) -> p g", p=P))
    nc.scalar.dma_start(out=bq_col, in_=bq_d.ap().rearrange("(g p) -> p g", p=P))
    nc.scalar.dma_start(out=gk_col, in_=gk_d.ap().rearrange("(g p) -> p g", p=P))
    nc.scalar.dma_start(out=bk_col, in_=bk_d.ap().rearrange("(g p) -> p g", p=P))
    for k in range(3):
        nc.sync.dma_start(out=xq_s[:, k, :], in_=xq_d.ap()[k * P:(k + 1) * P, :])
    for k in range(3):
        nc.gpsimd.dma_start(out=xk_s[:, k, :], in_=xk_d.ap()[k * P:(k + 1) * P, :])
    nc.gpsimd.dma_start(out=gv_col, in_=gv_d.ap().rearrange("(g p) -> p g", p=P))
    nc.gpsimd.dma_start(out=bv_col, in_=bv_d.ap().rearrange("(g p) -> p g", p=P))

    # ---------------- weight norms: scale = g / ||v|| as [128, 4] columns
    def emit_norms(w_nat, g_col, name):
        ss = tmp.tile([P, 4], F32, tag=f"ss_{name}")
        for g in range(4):
            sq = tmp.tile([P, KC], F32, tag="sq_shared")
            nc.vector.tensor_mul(sq, w_nat[:, g, :].bitcast(F32),
                                 w_nat[:, g, :].bitcast(F32))
            nc.vector.tensor_reduce(ss[:, g:g + 1], sq, axis=AX.X, op=ALU.add)
        r0 = tmp.tile([P, 4], F32, tag=f"r0_{name}")
        nc.scalar.activation(r0, ss, AF.Sqrt)
        nc.vector.reciprocal(r0, r0)
        h = tmp.tile([P, 4], F32, tag=f"h_{name}")
        nc.vector.tensor_mul(h, r0, r0)
        nc.vector.tensor_mul(h, h, ss)
        nc.vector.tensor_scalar(out=h, in0=h, scalar1=-0.5, scalar2=1.5,
                                op0=ALU.mult, op1=ALU.add)
        nc.vector.tensor_mul(r0, r0, h)  # refined rsqrt(ss)
        scale = singles.tile([P, 4], F32, tag=f"scale_{name}")
        nc.vector.tensor_mul(scale, g_col, r0)
        return scale

    # pure transposes: wt[:, k, 128g:128g+128] = (W[128g:.., 128k:..]).T
    def emit_transposes(w_nat, wt):
        for g in range(4):
            for k in range(3):
                pw = ps_x.tile([P, 512], F32, tag="mm")
                nc.tensor.matmul(
                    pw[:, :P].bitcast(F32R),
                    lhsT=w_nat[:, g, k * P:(k + 1) * P],
                    rhs=identr,
                    is_transpose=True,
                    start=True, stop=True,
                )
                nc.scalar.activation(wt[:, k, g * P:(g + 1) * P], pw[:, :P], AF.Copy)

    wt_q = singles.tile([P, 3, C], F32R, tag="wt_q")
    wt_k = singles.tile([P, 3, C], F32R, tag="wt_k")
    wt_v = singles.tile([P, 3, C], F32R, tag="wt_v")

    emit_transposes(w_nat_q, wt_q)
    emit_transposes(w_nat_k, wt_k)
    emit_transposes(w_nat_v, wt_v)
    ones_f32 = singles.tile([1, P], F32, tag="ones_f32")
    nc.vector.memset(ones_f32, 1.0)
    ones_row = singles.tile([1, P], F32R, tag="ones_row")
    nc.vector.tensor_copy(ones_row, ones_f32)
    qscale = emit_norms(w_nat_q, gq_col, "q")
    vscale = emit_norms(w_nat_v, gv_col, "v")

    # V: pre-divided bias b/s as a [1, 512] row (via column math + sb->sb DMA)
    bvs_col = singles.tile([P, 4], F32, tag="bvs_col")
    nc.vector.reciprocal(bvs_col, vscale)
    nc.vector.tensor_mul(bvs_col, bvs_col, bv_col)
    bvs_row = singles.tile([1, C], F32, tag="bvs_row")
    for g in range(4):
        nc.gpsimd.dma_start(
            out=bvs_row[0:1, g * P:(g + 1) * P], in_=bvs_col[:, g:g + 1]
        )
    bvs_rowr = singles.tile([1, C], F32R, tag="bvs_rowr")
    nc.vector.tensor_copy(bvs_rowr, bvs_row)
    # broadcast b/s to all partitions once; added during the yv evictions
    bias_psum = ps_o.tile([P, 512], F32, tag="po", name="bias_psum")
    nc.tensor.matmul(bias_psum, lhsT=ones_row, rhs=bvs_rowr, start=True, stop=True)
    bias_full = singles.tile([P, C], F32, tag="bias_full")
    nc.vector.tensor_copy(bias_full, bias_psum)
    kscale = emit_norms(w_nat_k, gk_col, "k")
    upper01 = singles.tile([P, P], BF16, tag="upper01")
    make_upper_triangular(nc, upper01, val=1.0, diag=False)

    # selector for the denominator broadcast: out[p] = srow[0] for p<64,
    # srow[32] for p>=64 (K=64 matmul, proven (64,128) PE tile shape)
    sel64f = singles.tile([D, P], F32, tag="sel64f")
    nc.vector.memset(sel64f, 0.0)
    nc.vector.memset(sel64f[0:1, 0:D], 1.0)
    nc.vector.memset(sel64f[32:33, D:P], 1.0)
    sel64 = singles.tile([D, P], F32R, tag="sel64")
    nc.vector.tensor_copy(sel64, sel64f)
    srow_ab = []
    for nm in ("srow_a", "srow_b"):
        t = singles.tile([D, 512], F32R, tag=nm, name=nm)
        nc.vector.memset(t.bitcast(mybir.dt.uint32), 0)
        srow_ab.append(t)



    # ---------------- Q/K projections: [c%128, c//128, s] bf16
    qt = singles.tile([P, 4, S], BF16, tag="qt")
    kt = singles.tile([P, 4, S], BF16, tag="kt")

    def emit_proj(g):
        for dst, wt, scol, bcol, xs in (
            (qt, wt_q, qscale, bq_col, xq_s),
            (kt, wt_k, kscale, bk_col, xk_s),
        ):
            for j in range(2):
                pp = ps_x.tile([P, 512], F32, tag="mm")
                for k in range(3):
                    nc.tensor.matmul(
                        pp,
                        lhsT=wt[:, k, g * P:(g + 1) * P],
                        rhs=xs[:, k, j * 512:(j + 1) * 512],
                        start=(k == 0),
                        stop=(k == 2),
                    )
                nc.scalar.activation(
                    dst[:, g, j * 512:(j + 1) * 512], pp, AF.Identity,
                    bias=bcol[:, g:g + 1], scale=scol[:, g:g + 1],
                )

    # ---------------- V projection, [S, C] bf16, heads strided by 65
    # yv[:, t, h, 0:64] = raw V + (b/s); yv[:, t, h, 64] = 1 (denominator col)
    yv = singles.tile([P, 8, NH, 65], BF16, tag="yv")
    nc.gpsimd.memset(yv[:, :, :, 64:65], 1.0)

    def emit_vproj():
        for t in range(8):
            pv = ps_x.tile([P, 512], F32, tag="mm")
            for k in range(3):
                nc.tensor.matmul(
                    pv,
                    lhsT=xk_s[:, k, t * P:(t + 1) * P],
                    rhs=wt_v[:, k, :],
                    start=(k == 0),
                    stop=(k == 2),
                )
            nc.vector.tensor_add(
                yv[:, t, :, 0:64],
                pv[:, :].rearrange("p (h d) -> p h d", h=NH),
                bias_full[:, :].rearrange("p (h d) -> p h d", h=NH),
            )

    # ---------------- attention
    def emit_group(g4, j, pending):
        """Emit one (g4, j) score/AV group; the previous group's normalize
        (pending) is flushed after this group's first score pair so the PE
        queue is never head-of-line blocked on the normalize chain."""
        n_i = 4 * j + 4
        po = {}
        for half in (0, 1):
            po[half] = ps_o.tile([P, 512], F32, tag="po", name="po")
        es_tiles = {}
        pst = {}

        def r0_of(i):
            return P * max(i - 4 * j, 0)

        def emit_score_pair(i):
            r0 = r0_of(i)
            for half in (0, 1):
                pr = slice(D * half, D * half + D)
                p = ps_s.tile([P, 512], F32, tag="mm")
                nc.tensor.matmul(
                    p[:, r0:],
                    lhsT=kt[pr, g4, i * P:(i + 1) * P],
                    rhs=qt[pr, g4, 512 * j + r0:512 * (j + 1)],
                    start=True, stop=True,
                )
                pst[(half, i)] = p

        def emit_evict(i):
            r0 = r0_of(i)
            for half in (0, 1):
                et = es_pool.tile([P, 512], BF16, tag="es")
                if half == 0:
                    nc.scalar.activation(
                        et[:, r0:], pst[(half, i)][:, r0:], AF.Exp, scale=0.125
                    )
                else:
                    # Schraudolph fast-exp: i16 = s*A + B, bitcast bf16
                    nc.vector.tensor_scalar(
                        out=et[:, r0:].bitcast(I16),
                        in0=pst[(half, i)][:, r0:],
                        scalar1=EXP_A, scalar2=EXP_B,
                        op0=ALU.mult, op1=ALU.add,
                    )
                if i - 4 * j >= 0:
                    nc.gpsimd.tensor_mul(
                        et[:, r0:r0 + P], et[:, r0:r0 + P], upper01
                    )
                es_tiles[(half, i)] = et

        def emit_av(i):
            r0 = r0_of(i)
            for half in (0, 1):
                h = 2 * g4 + half
                nc.tensor.matmul(
                    po[half][0:65, r0:],
                    lhsT=yv[:, i, h, :],
                    rhs=es_tiles[(half, i)][:, r0:],
                    start=(i == 0),
                    stop=(i == n_i - 1),
                )

        emit_score_pair(0)
        if pending is not None:
            pending()
        for i in range(n_i):
            if i + 1 < n_i:
                emit_score_pair(i + 1)
            emit_evict(i)
            emit_av(i)

        def finalize():
            # normalize + output: one K=64 selector matmul broadcasts both
            # halves' denominator rows into a full [128, 512] bank.
            srow = srow_ab[(2 * g4 + j) % 2]
            for half in (0, 1):
                nc.vector.tensor_scalar(
                    out=srow[32 * half:32 * half + 1, :],
                    in0=po[half][64:65, :],
                    scalar1=1e-30, scalar2=None, op0=ALU.max,
                )
            pbt = ps_x.tile([P, 512], F32, tag="mm")
            nc.tensor.matmul(
                pbt, lhsT=sel64, rhs=srow, start=True, stop=True,
            )
            bb = small.tile([P, 512], F32, tag="bb")
            nc.vector.reciprocal_approx_fast(bb, pbt)
            for half in (0, 1):
                h = 2 * g4 + half
                ot = out_pool.tile([D, 512], F32, tag="ot")
                nc.vector.scalar_tensor_tensor(
                    out=ot, in0=po[half][0:64, :],
                    scalar=vscale[D * half:D * half + D, g4:g4 + 1],
                    in1=bb[D * half:D * half + D, :],
                    op0=ALU.mult, op1=ALU.mult,
                )
                nc.sync.dma_start(
                    out=out_d.ap()[D * h:D * h + D, 512 * j:512 * (j + 1)],
                    in_=ot,
                )

        return finalize

    emit_proj(0)
    emit_vproj()
    pending = None
    for g4 in range(4):
        for j in (0, 1):
            pending = emit_group(g4, j, pending)
        if g4 + 1 < 4:
            emit_proj(g4 + 1)
    pending()


_NC_CACHE = {}


def _get_nc():
    if "nc" not in _NC_CACHE:
        _NC_CACHE["nc"] = build_nc()
    return _NC_CACHE["nc"]


def kernel(q, k, Wq_v, Wq_g, bq, Wk_v, Wk_g, bk, Wv_v, Wv_g, bv, trace=False):
    nc = _get_nc()
    q = np.asarray(q, np.float32)
    k = np.asarray(k, np.float32)
    common = {
        "wq": np.ascontiguousarray(np.asarray(Wq_v, np.float32)),
        "wk": np.ascontiguousarray(np.asarray(Wk_v, np.float32)),
        "wv": np.ascontiguousarray(np.asarray(Wv_v, np.float32)),
        "gq": np.ascontiguousarray(np.asarray(Wq_g, np.float32)),
        "gk": np.ascontiguousarray(np.asarray(Wk_g, np.float32)),
        "gv": np.ascontiguousarray(np.asarray(Wv_g, np.float32)),
        "bq": np.ascontiguousarray(np.asarray(bq, np.float32)),
        "bk": np.ascontiguousarray(np.asarray(bk, np.float32)),
        "bv": np.ascontiguousarray(np.asarray(bv, np.float32)),
    }
    in_maps = []
    for b in range(NB):
        m = dict(common)
        m["xq"] = np.ascontiguousarray(q[b].reshape(KC, S))
        m["xk"] = np.ascontiguousarray(k[b].reshape(KC, S))
        in_maps.append(m)
    res = run_bass_kernel_spmd(nc, in_maps, core_ids=list(range(NB)), trace=trace)
    out = np.stack([res.results[b]["out"] for b in range(NB)])  # [8, 512, 1024]
    out = out.reshape(NB, C, 32, 32).astype(np.float32)
    if trace:
        kernel.last_results = res
    return out


# revision 37
# speedup vs baseline: 1.0389x; 1.0389x over previous
"""Trainium2 Bass kernel for nn_CausalAttention_56873956934253.

Causal attention, B=8, S=1024 (32x32), C=512, 8 heads, D=64, with
weight-normalized QKV projections (PyTorch weight_norm dim=0 style).

Sharding: pure data parallelism over batch - core b handles batch b.
Weights replicated. No collectives.

Per-core dataflow (all shapes hardcoded):
  WT prep:  pure PE transposes (fp32r is_transpose, 1.5 cyc/row) of the three
            natural [C, KC] weights into [KC%128, 3, C] layout.  The
            weight-norm scale g/||v|| is NOT applied to the weights:
            - Q, K: scale+bias folded into the PSUM->SBUF eviction of the
              projections (ACT Identity with per-partition scale/bias APs),
              output directly in bf16.
            - V: scale folded into the final normalization multiply
              (scalar_tensor_tensor per-partition scalar), bias pre-divided
              by the scale and added to raw V via a K=1 ones matmul.
  norms:    fused square+reduce (tensor_tensor_reduce) + exact sqrt/recip
            with one Newton refinement.
  QT/KT:    [C, S] layout bf16 (PE fp32r matmuls over 3 K-chunks).
  Yv:       [S, C] layout bf16, heads strided by 65 with a ones column
            (AV matmul then also produces softmax denominators).
  Scores:   bf16, per head pair on disjoint 64-row PE groups, emitted as
            adjacent pairs with 1-step software pipelining against the
            AV matmuls.  exp(s/8) fused into ACT eviction; strictly-upper
            diag blocks masked multiplicatively post-exp (DVE for half 0,
            GPSIMD for half 1).
  AV:       bf16, V stationary, accumulating po[65, q]; row 64 = softmax
            denominators.  Denominator row -> guard-max to SBUF -> K=1
            broadcast matmul into partitions 64:128 of the SAME po bank
            (no extra PSUM) -> reciprocal_approx_fast -> fused
            (po * v_scale) * recip normalize -> DMA out per (head, j).
  DMA:      inputs spread over 5 queues (sync/scalar/tensor/vector/gpsimd).
  Output [C, S] per core -> [8, 512, 32, 32].
"""

import numpy as np
from contextlib import ExitStack

import concourse.bacc as bacc
import concourse.bass as bass
import concourse.tile as tile
import concourse.mybir as mybir
from concourse.bass_utils import run_bass_kernel_spmd
from concourse.masks import make_identity, make_upper_triangular

P = 128
S = 1024
C = 512
KC = 384
NH = 8
D = 64
NB = 8  # batch == cores

F32 = mybir.dt.float32
F32R = mybir.dt.float32r
BF16 = mybir.dt.bfloat16

I16 = mybir.dt.int16
# fast-exp constants: exp(s/8) ~= bf16_frombits(round(s*A + B))
EXP_A = 0.125 * 1.4426950408889634 * 128.0
EXP_B = 16256.0 - 5.5

AF = mybir.ActivationFunctionType
ALU = mybir.AluOpType
AX = mybir.AxisListType


def _r(ap):
    return ap.bitcast(F32R)


def build_nc():
    nc = bacc.Bacc("TRN2", target_bir_lowering=False, debug=False)

    xq_d = nc.dram_tensor("xq", [KC, S], F32R, kind="ExternalInput")
    xk_d = nc.dram_tensor("xk", [KC, S], F32R, kind="ExternalInput")
    wq_d = nc.dram_tensor("wq", [C, KC], F32R, kind="ExternalInput")
    wk_d = nc.dram_tensor("wk", [C, KC], F32R, kind="ExternalInput")
    wv_d = nc.dram_tensor("wv", [C, KC], F32R, kind="ExternalInput")
    gq_d = nc.dram_tensor("gq", [C], F32, kind="ExternalInput")
    gk_d = nc.dram_tensor("gk", [C], F32, kind="ExternalInput")
    gv_d = nc.dram_tensor("gv", [C], F32, kind="ExternalInput")
    bq_d = nc.dram_tensor("bq", [C], F32, kind="ExternalInput")
    bk_d = nc.dram_tensor("bk", [C], F32, kind="ExternalInput")
    bv_d = nc.dram_tensor("bv", [C], F32, kind="ExternalInput")
    out_d = nc.dram_tensor("out", [C, S], F32, kind="ExternalOutput")

    with tile.TileContext(nc) as tc:
        with ExitStack() as ctx:
            _body(ctx, tc, xq_d, xk_d,
                  (wq_d, gq_d, bq_d), (wk_d, gk_d, bk_d), (wv_d, gv_d, bv_d),
                  out_d)
    nc.compile()
    return nc


def _body(ctx, tc, xq_d, xk_d, wq3, wk3, wv3, out_d):
    nc = tc.nc

    singles = ctx.enter_context(tc.tile_pool(name="singles", bufs=1))
    tmp = ctx.enter_context(tc.tile_pool(name="tmp", bufs=2))
    ps_s = ctx.enter_context(tc.tile_pool(name="ps_s", bufs=3, space="PSUM"))
    ps_x = ctx.enter_context(tc.tile_pool(name="ps_x", bufs=2, space="PSUM"))
    ps_o = ctx.enter_context(tc.tile_pool(name="ps_o", bufs=3, space="PSUM"))
    es_pool = ctx.enter_context(tc.tile_pool(name="es", bufs=28))
    out_pool = ctx.enter_context(tc.tile_pool(name="outp", bufs=4))
    small = ctx.enter_context(tc.tile_pool(name="small", bufs=8))

    wq_d, gq_d, bq_d = wq3
    wk_d, gk_d, bk_d = wk3
    wv_d, gv_d, bv_d = wv3

    # ---------------- input DMAs, spread across the 5 engine queues
    w_nat_q = singles.tile([P, 4, KC], F32R, tag="wnq")
    w_nat_k = singles.tile([P, 4, KC], F32R, tag="wnk")
    w_nat_v = singles.tile([P, 4, KC], F32R, tag="wnv")
    xq_s = singles.tile([P, 3, S], F32R, tag="xq_s")
    xk_s = singles.tile([P, 3, S], F32R, tag="xk_s")
    # per-channel vectors in [128, 4] column layout (c = g*128 + p)
    gq_col = singles.tile([P, 4], F32, tag="gq")
    bq_col = singles.tile([P, 4], F32, tag="bq")
    gk_col = singles.tile([P, 4], F32, tag="gk")
    bk_col = singles.tile([P, 4], F32, tag="bk")
    gv_col = singles.tile([P, 4], F32, tag="gv")
    bv_col = singles.tile([P, 4], F32, tag="bv")

    # tiny gathers first (~1us each while HBM is still quiet), then the big
    # streams; wq gets the full early bandwidth on the sync queue.
    for g in range(4):
        nc.sync.dma_start(out=w_nat_q[:, g, :], in_=wq_d.ap()[g * P:(g + 1) * P, :])
    for g in range(4):
        nc.scalar.dma_start(out=w_nat_k[:, g, :], in_=wk_d.ap()[g * P:(g + 1) * P, :])
    for g in range(4):
        nc.gpsimd.dma_start(out=w_nat_v[:, g, :], in_=wv_d.ap()[g * P:(g + 1) * P, :])
    nc.scalar.dma_start(out=gq_col, in_=gq_d.ap().rearrange("(g p) -> p g", p=P))
    nc.scalar.dma_start(out=bq_col, in_=bq_d.ap().rearrange("(g p) -> p g", p=P))
    nc.scalar.dma_start(out=gk_col, in_=gk_d.ap().rearrange("(g p) -> p g", p=P))
    nc.scalar.dma_start(out=bk_col, in_=bk_d.ap().rearrange("(g p) -> p g", p=P))
    for k in range(3):
        nc.sync.dma_start(out=xq_s[:, k, :], in_=xq_d.ap()[k * P:(k + 1) * P, :])
    for k in range(3):
        nc.gpsimd.dma_start(out=xk_s[:, k, :], in_=xk_d.ap()[k * P:(k + 1) * P, :])
    nc.gpsimd.dma_start(out=gv_col, in_=gv_d.ap().rearrange("(g p) -> p g", p=P))
    nc.gpsimd.dma_start(out=bv_col, in_=bv_d.ap().rearrange("(g p) -> p g", p=P))

    # ---------------- constants (only ident/identr before the transposes so
    # the PE's first op doesn't pace on unrelated DVE setup work)
    ident = singles.tile([P, P], F32, tag="ident")
    make_identity(nc, ident)
    identr = singles.tile([P, P], F32R, tag="identr")
    nc.vector.tensor_copy(identr, ident)

    # ---------------- weight norms: scale = g / ||v|| as [128, 4] columns
    def emit_norms(w_nat, g_col, name):
        ss = tmp.tile([P, 4], F32, tag=f"ss_{name}")
        for g in range(4):
            sq = tmp.tile([P, KC], F32, tag="sq_shared")
            nc.vector.tensor_mul(sq, w_nat[:, g, :].bitcast(F32),
                                 w_nat[:, g, :].bitcast(F32))
            nc.vector.tensor_reduce(ss[:, g:g + 1], sq, axis=AX.X, op=ALU.add)
        r0 = tmp.tile([P, 4], F32, tag=f"r0_{name}")
        nc.scalar.activation(r0, ss, AF.Sqrt)
        nc.vector.reciprocal(r0, r0)
        h = tmp.tile([P, 4], F32, tag=f"h_{name}")
        nc.vector.tensor_mul(h, r0, r0)
        nc.vector.tensor_mul(h, h, ss)
        nc.vector.tensor_scalar(out=h, in0=h, scalar1=-0.5, scalar2=1.5,
                                op0=ALU.mult, op1=ALU.add)
        nc.vector.tensor_mul(r0, r0, h)  # refined rsqrt(ss)
        scale = singles.tile([P, 4], F32, tag=f"scale_{name}")
        nc.vector.tensor_mul(scale, g_col, r0)
        return scale

    # pure transposes: wt[:, k, 128g:128g+128] = (W[128g:.., 128k:..]).T
    def emit_transposes(w_nat, wt):
        for g in range(4):
            for k in range(3):
                pw = ps_x.tile([P, 512], F32, tag="mm")
                nc.tensor.matmul(
                    pw[:, :P].bitcast(F32R),
                    lhsT=w_nat[:, g, k * P:(k + 1) * P],
                    rhs=identr,
                    is_transpose=True,
                    start=True, stop=True,
                )
                nc.scalar.activation(wt[:, k, g * P:(g + 1) * P], pw[:, :P], AF.Copy)

    wt_q = singles.tile([P, 3, C], F32R, tag="wt_q")
    wt_k = singles.tile([P, 3, C], F32R, tag="wt_k")
    wt_v = singles.tile([P, 3, C], F32R, tag="wt_v")

    emit_transposes(w_nat_q, wt_q)
    emit_transposes(w_nat_k, wt_k)
    emit_transposes(w_nat_v, wt_v)
    ones_f32 = singles.tile([1, P], F32, tag="ones_f32")
    nc.vector.memset(ones_f32, 1.0)
    ones_row = singles.tile([1, P], F32R, tag="ones_row")
    nc.vector.tensor_copy(ones_row, ones_f32)
    qscale = emit_norms(w_nat_q, gq_col, "q")
    vscale = emit_norms(w_nat_v, gv_col, "v")

    # V: pre-divided bias b/s as a [1, 512] row (via column math + sb->sb DMA)
    bvs_col = singles.tile([P, 4], F32, tag="bvs_col")
    nc.vector.reciprocal(bvs_col, vscale)
    nc.vector.tensor_mul(bvs_col, bvs_col, bv_col)
    bvs_row = singles.tile([1, C], F32, tag="bvs_row")
    for g in range(4):
        nc.gpsimd.dma_start(
            out=bvs_row[0:1, g * P:(g + 1) * P], in_=bvs_col[:, g:g + 1]
        )
    bvs_rowr = singles.tile([1, C], F32R, tag="bvs_rowr")
    nc.vector.tensor_copy(bvs_rowr, bvs_row)
    # broadcast b/s to all partitions once; added during the yv evictions
    bias_psum = ps_o.tile([P, 512], F32, tag="po", name="bias_psum")
    nc.tensor.matmul(bias_psum, lhsT=ones_row, rhs=bvs_rowr, start=True, stop=True)
    bias_full = singles.tile([P, C], F32, tag="bias_full")
    nc.vector.tensor_copy(bias_full, bias_psum)
    kscale = emit_norms(w_nat_k, gk_col, "k")
    upper01 = singles.tile([P, P], BF16, tag="upper01")
    make_upper_triangular(nc, upper01, val=1.0, diag=False)

    # selector for the denominator broadcast: out[p] = srow[0] for p<64,
    # srow[32] for p>=64 (K=64 matmul, proven (64,128) PE tile shape)
    sel64f = singles.tile([D, P], F32, tag="sel64f")
    nc.vector.memset(sel64f, 0.0)
    nc.vector.memset(sel64f[0:1, 0:D], 1.0)
    nc.vector.memset(sel64f[32:33, D:P], 1.0)
    sel64 = singles.tile([D, P], F32R, tag="sel64")
    nc.vector.tensor_copy(sel64, sel64f)
    srow_ab = []
    for nm in ("srow_a", "srow_b"):
        t = singles.tile([D, 512], F32R, tag=nm, name=nm)
        nc.vector.memset(t.bitcast(mybir.dt.uint32), 0)
        srow_ab.append(t)



    # ---------------- Q/K projections: [c%128, c//128, s] bf16
    qt = singles.tile([P, 4, S], BF16, tag="qt")
    kt = singles.tile([P, 4, S], BF16, tag="kt")

    def emit_proj(g):
        for dst, wt, scol, bcol, xs in (
            (qt, wt_q, qscale, bq_col, xq_s),
            (kt, wt_k, kscale, bk_col, xk_s),
        ):
            for j in range(2):
                pp = ps_x.tile([P, 512], F32, tag="mm")
                for k in range(3):
                    nc.tensor.matmul(
                        pp,
                        lhsT=wt[:, k, g * P:(g + 1) * P],
                        rhs=xs[:, k, j * 512:(j + 1) * 512],
                        start=(k == 0),
                        stop=(k == 2),
                    )
                nc.scalar.activation(
                    dst[:, g, j * 512:(j + 1) * 512], pp, AF.Identity,
                    bias=bcol[:, g:g + 1], scale=scol[:, g:g + 1],
                )

    # ---------------- V projection, [S, C] bf16, heads strided by 65
    # yv[:, t, h, 0:64] = raw V + (b/s); yv[:, t, h, 64] = 1 (denominator col)
    yv = singles.tile([P, 8, NH, 65], BF16, tag="yv")
    nc.gpsimd.memset(yv[:, :, :, 64:65], 1.0)

    def emit_vproj():
        for t in range(8):
            pv = ps_x.tile([P, 512], F32, tag="mm")
            for k in range(3):
                nc.tensor.matmul(
                    pv,
                    lhsT=xk_s[:, k, t * P:(t + 1) * P],
                    rhs=wt_v[:, k, :],
                    start=(k == 0),
                    stop=(k == 2),
                )
            nc.vector.tensor_add(
                yv[:, t, :, 0:64],
                pv[:, :].rearrange("p (h d) -> p h d", h=NH),
                bias_full[:, :].rearrange("p (h d) -> p h d", h=NH),
            )

    # ---------------- attention
    def emit_group(g4, j, pending):
        """Emit one (g4, j) score/AV group; the previous group's normalize
        (pending) is flushed after this group's first score pair so the PE
        queue is never head-of-line blocked on the normalize chain."""
        n_i = 4 * j + 4
        po = {}
        for half in (0, 1):
            po[half] = ps_o.tile([P, 512], F32, tag="po", name="po")
        es_tiles = {}
        pst = {}

        def r0_of(i):
            return P * max(i - 4 * j, 0)

        def emit_score_pair(i):
            r0 = r0_of(i)
            for half in (0, 1):
                pr = slice(D * half, D * half + D)
                p = ps_s.tile([P, 512], F32, tag="mm")
                nc.tensor.matmul(
                    p[:, r0:],
                    lhsT=kt[pr, g4, i * P:(i + 1) * P],
                    rhs=qt[pr, g4, 512 * j + r0:512 * (j + 1)],
                    start=True, stop=True,
                )
                pst[(half, i)] = p

        def emit_evict(i):
            r0 = r0_of(i)
            for half in (0, 1):
                et = es_pool.tile([P, 512], BF16, tag="es")
                if half == 0:
                    nc.scalar.activation(
                        et[:, r0:], pst[(half, i)][:, r0:], AF.Exp, scale=0.125
                    )
                else:
                    # Schraudolph fast-exp: i16 = s*A + B, bitcast bf16
                    nc.vector.tensor_scalar(
                        out=et[:, r0:].bitcast(I16),
                        in0=pst[(half, i)][:, r0:],
                        scalar1=EXP_A, scalar2=EXP_B,
                        op0=ALU.mult, op1=ALU.add,
                    )
                if i - 4 * j >= 0:
                    nc.gpsimd.tensor_mul(
                        et[:, r0:r0 + P], et[:, r0:r0 + P], upper01
                    )
                es_tiles[(half, i)] = et

        def emit_av(i):
            r0 = r0_of(i)
            for half in (0, 1):
                h = 2 * g4 + half
                nc.tensor.matmul(
                    po[half][0:65, r0:],
                    lhsT=yv[:, i, h, :],
                    rhs=es_tiles[(half, i)][:, r0:],
                    start=(i == 0),
                    stop=(i == n_i - 1),
                )

        emit_score_pair(0)
        if pending is not None:
            pending()
        for i in range(n_i):
            if i + 1 < n_i:
                emit_score_pair(i + 1)
            emit_evict(i)
            emit_av(i)

        def finalize():
            # normalize + output: one K=64 selector matmul broadcasts both
            # halves' denominator rows into a full [128, 512] bank.
            srow = srow_ab[(2 * g4 + j) % 2]
            for half in (0, 1):
                nc.vector.tensor_scalar(
                    out=srow[32 * half:32 * half + 1, :],
                    in0=po[half][64:65, :],
                    scalar1=1e-30, scalar2=None, op0=ALU.max,
                )
            pbt = ps_x.tile([P, 512], F32, tag="mm")
            nc.tensor.matmul(
                pbt, lhsT=sel64, rhs=srow, start=True, stop=True,
            )
            bb = small.tile([P, 512], F32, tag="bb")
            nc.vector.reciprocal_approx_fast(bb, pbt)
            for half in (0, 1):
                h = 2 * g4 + half
                ot = out_pool.tile([D, 512], F32, tag="ot")
                nc.vector.scalar_tensor_tensor(
                    out=ot, in0=po[half][0:64, :],
                    scalar=vscale[D * half:D * half + D, g4:g4 + 1],
                    in1=bb[D * half:D * half + D, :],
                    op0=ALU.mult, op1=ALU.mult,
                )
                nc.sync.dma_start(
                    out=out_d.ap()[D * h:D * h + D, 512 * j:512 * (j + 1)],
                    in_=ot,
                )

        return finalize

    emit_proj(0)
    emit_vproj()
    pending = None
    for g4 in range(4):
        for j in (0, 1):
            pending = emit_group(g4, j, pending)
        if g4 + 1 < 4:
            emit_proj(g4 + 1)
    pending()


_NC_CACHE = {}


def _get_nc():
    if "nc" not in _NC_CACHE:
        _NC_CACHE["nc"] = build_nc()
    return _NC_CACHE["nc"]


def kernel(q, k, Wq_v, Wq_g, bq, Wk_v, Wk_g, bk, Wv_v, Wv_g, bv, trace=False):
    nc = _get_nc()
    q = np.asarray(q, np.float32)
    k = np.asarray(k, np.float32)
    common = {
        "wq": np.ascontiguousarray(np.asarray(Wq_v, np.float32)),
        "wk": np.ascontiguousarray(np.asarray(Wk_v, np.float32)),
        "wv": np.ascontiguousarray(np.asarray(Wv_v, np.float32)),
        "gq": np.ascontiguousarray(np.asarray(Wq_g, np.float32)),
        "gk": np.ascontiguousarray(np.asarray(Wk_g, np.float32)),
        "gv": np.ascontiguousarray(np.asarray(Wv_g, np.float32)),
        "bq": np.ascontiguousarray(np.asarray(bq, np.float32)),
        "bk": np.ascontiguousarray(np.asarray(bk, np.float32)),
        "bv": np.ascontiguousarray(np.asarray(bv, np.float32)),
    }
    in_maps = []
    for b in range(NB):
        m = dict(common)
        m["xq"] = np.ascontiguousarray(q[b].reshape(KC, S))
        m["xk"] = np.ascontiguousarray(k[b].reshape(KC, S))
        in_maps.append(m)
    res = run_bass_kernel_spmd(nc, in_maps, core_ids=list(range(NB)), trace=trace)
    out = np.stack([res.results[b]["out"] for b in range(NB)])  # [8, 512, 1024]
    out = out.reshape(NB, C, 32, 32).astype(np.float32)
    if trace:
        kernel.last_results = res
    return out
